# revision 5
# baseline (speedup 1.0000x reference)
"""Trainium2 Bass kernel for MetaBayesLinearParallel.

Math (per sample s):
    W[s]  = weight_mu + weight_sigma * eps_w[s]          # (OUT, IN)
    Bv[s] = bias_mu + bias_sigma * eps_b[s]              # (OUT,)
    out[s] = x[s] @ W[s].T + Bv[s]                       # (B, OUT)

Sharding over 8 cores: 2-way split of the samples axis x 4-way split of
OUT.  Each core handles S_PC=4 samples and O_PC=512 output rows, which
minimizes per-core HBM traffic (16MB eps + 8MB x + 8MB mu/sigma = 32MB).

Per-core pipeline (all compute in bf16, fp32 PSUM accumulation):
  once:  sigma tiles cast-loaded;  muT = transpose(mu) built via PE
  per sample:
    xT   = transpose(x[s])                        (PE transpose + ACT copy)
    se   = sigma * eps_w[s]                       (DVE, bf16 2x mode)
    WT_i = transpose(se)_i + muT_i                (PE transpose + DVE add)
    psum[b,:] = sum_i xT_i[:,b].T @ WT_i  (+ ones.T @ Bv via K=1 matmul)
    out[s,b,:] = psum                             (ACT copy + DMA store)
"""

from contextlib import ExitStack

import numpy as np

import concourse.bacc as bacc
import concourse.mybir as mybir
import concourse.tile as tile
from concourse.bass_utils import run_bass_kernel_spmd
from concourse.masks import make_identity

P = 128
S, B, IN, OUT = 8, 256, 2048, 2048
SAMPLE_WAYS, OUT_WAYS = 2, 4
N_CORES = SAMPLE_WAYS * OUT_WAYS
S_PC = S // SAMPLE_WAYS
O_PC = OUT // OUT_WAYS

BF16 = mybir.dt.bfloat16
F32 = mybir.dt.float32


def build_core_program(s_pc=S_PC, o_pc=O_PC, in_dim=IN, b_dim=B):
    """One NeuronCore's program; identical on all cores (SPMD over slices)."""
    o_tiles = o_pc // P
    i_blks = in_dim // P
    b_tiles = b_dim // P

    nc = bacc.Bacc("TRN2")
    x_d = nc.declare_dram_parameter("x", [s_pc, b_dim, in_dim], F32, isOutput=False)
    eps_d = nc.declare_dram_parameter("eps_w", [s_pc, o_pc, in_dim], F32, isOutput=False)
    mu_d = nc.declare_dram_parameter("mu", [o_pc, in_dim], F32, isOutput=False)
    sig_d = nc.declare_dram_parameter("sigma", [o_pc, in_dim], F32, isOutput=False)
    bmu_d = nc.declare_dram_parameter("bias_mu", [1, o_pc], F32, isOutput=False)
    bsig_d = nc.declare_dram_parameter("bias_sigma", [1, o_pc], F32, isOutput=False)
    epsb_d = nc.declare_dram_parameter("eps_b", [s_pc, o_pc], F32, isOutput=False)
    out_d = nc.declare_dram_parameter("out", [s_pc, b_dim, o_pc], F32, isOutput=True)

    with ExitStack() as ctx:
        tc = ctx.enter_context(tile.TileContext(nc))
        consts = ctx.enter_context(tc.tile_pool(name="consts", bufs=1))
        resident = ctx.enter_context(tc.tile_pool(name="resident", bufs=1))
        ld = ctx.enter_context(tc.tile_pool(name="ld", bufs=3))
        work = ctx.enter_context(tc.tile_pool(name="work", bufs=2))
        wt_pool = ctx.enter_context(tc.tile_pool(name="wt", bufs=3))
        outp = ctx.enter_context(tc.tile_pool(name="outp", bufs=3))
        ps_tr = ctx.enter_context(tc.tile_pool(name="ps_tr", bufs=2, space="PSUM"))
        ps_xt = ctx.enter_context(tc.tile_pool(name="ps_xt", bufs=2, space="PSUM"))
        ps_out = ctx.enter_context(tc.tile_pool(name="ps_out", bufs=2, space="PSUM"))

        ident = consts.tile([P, P], BF16)
        make_identity(nc, ident)
        ones = consts.tile([1, P], BF16)
        nc.vector.memset(ones[:], 1.0)

        # ---- resident tensors ----
        sigma_sb = resident.tile([P, o_tiles, in_dim], BF16)
        for ot in range(o_tiles):
            nc.gpsimd.dma_start(out=sigma_sb[:, ot, :], in_=sig_d[ot * P:(ot + 1) * P, :])

        # muT[i, o] built once via PE transpose of cast-loaded mu tiles
        muT_sb = resident.tile([P, i_blks, o_pc], BF16)
        for ot in range(o_tiles):
            mu_t = ld.tile([P, in_dim], BF16, tag="mu_ld")
            nc.gpsimd.dma_start(out=mu_t[:], in_=mu_d[ot * P:(ot + 1) * P, :])
            for ib in range(i_blks):
                pmu = ps_tr.tile([P, P], BF16, tag="ps_mu")
                nc.tensor.transpose(pmu[:], mu_t[:, ib * P:(ib + 1) * P], ident[:])
                nc.scalar.copy(muT_sb[:, ib, ot * P:(ot + 1) * P], pmu[:])

        # bias inputs (tiny)
        bmu_sb = consts.tile([1, o_pc], F32)
        nc.sync.dma_start(out=bmu_sb[:], in_=bmu_d[:, :])
        bsig_sb = consts.tile([1, o_pc], F32)
        nc.sync.dma_start(out=bsig_sb[:], in_=bsig_d[:, :])
        epsb_sb = consts.tile([1, s_pc * o_pc], F32)
        nc.sync.dma_start(out=epsb_sb[:], in_=epsb_d[:, :])

        for s in range(s_pc):
            # ---- xT[s]: [in, b] via PE transpose ----
            xb = work.tile([P, b_tiles, in_dim], BF16, tag="xb")
            for bt in range(b_tiles):
                nc.gpsimd.dma_start(out=xb[:, bt, :], in_=x_d[s, bt * P:(bt + 1) * P, :])
            xT = work.tile([P, i_blks, b_dim], BF16, tag="xT")
            for ib in range(i_blks):
                pxt = ps_xt.tile([P, b_dim], BF16, tag="ps_xt")
                for bt in range(b_tiles):
                    nc.tensor.transpose(
                        pxt[:, bt * P:(bt + 1) * P], xb[:, bt, ib * P:(ib + 1) * P], ident[:])
                nc.scalar.copy(xT[:, ib, :], pxt[:])

            # ---- se = sigma * eps[s] (bf16, natural [o, i] layout) ----
            se = work.tile([P, o_tiles, in_dim], BF16, tag="se")
            for ot in range(o_tiles):
                eps_t = ld.tile([P, in_dim], BF16, tag="eps_ld")
                nc.gpsimd.dma_start(
                    out=eps_t[:], in_=eps_d[s, ot * P:(ot + 1) * P, :])
                nc.vector.tensor_mul(se[:, ot, :], eps_t[:], sigma_sb[:, ot, :])

            # ---- bias vector Bv[s] (bf16 [1, o_pc]) ----
            btmp = ld.tile([1, o_pc], F32, tag="btmp")
            nc.vector.tensor_mul(btmp[:], bsig_sb[:], epsb_sb[:, s * o_pc:(s + 1) * o_pc])
            bv = ld.tile([1, o_pc], BF16, tag="bv")
            nc.vector.tensor_add(bv[:], bmu_sb[:], btmp[:])

            # ---- per i-block: WT_i = transpose(se)_i + muT_i, then matmuls ----
            psum_out = []
            for bt in range(b_tiles):
                po = ps_out.tile([P, o_pc], F32, tag="ps_out", name=f"ps_out_{s}_{bt}")
                psum_out.append(po)
            for ib in range(i_blks):
                pseT = ps_tr.tile([P, o_pc], BF16, tag="ps_seT")
                for ot in range(o_tiles):
                    nc.tensor.transpose(
                        pseT[:, ot * P:(ot + 1) * P], se[:, ot, ib * P:(ib + 1) * P], ident[:])
                wt = wt_pool.tile([P, o_pc], BF16, tag="wt")
                nc.vector.tensor_add(wt[:], pseT[:], muT_sb[:, ib, :])
                for bt in range(b_tiles):
                    nc.tensor.matmul(
                        psum_out[bt][:], xT[:, ib, bt * P:(bt + 1) * P], wt[:],
                        start=(ib == 0), stop=False)
            for bt in range(b_tiles):
                nc.tensor.matmul(psum_out[bt][:], ones[:], bv[:], start=False, stop=True)
                o_sb = outp.tile([P, o_pc], F32, tag="o_sb")
                nc.scalar.copy(o_sb[:], psum_out[bt][:])
                nc.sync.dma_start(out=out_d[s, bt * P:(bt + 1) * P, :], in_=o_sb[:])

    nc.compile()
    return nc


_prog_cache = {}
_last_in_maps = None


def _get_program(key):
    if key not in _prog_cache:
        _prog_cache[key] = build_core_program(*key)
    return _prog_cache[key]


def kernel(x, weight_mu, weight_sigma, bias_mu, bias_sigma, eps_w, eps_b):
    x = np.ascontiguousarray(x, dtype=np.float32)
    weight_mu = np.ascontiguousarray(weight_mu, dtype=np.float32)
    weight_sigma = np.ascontiguousarray(weight_sigma, dtype=np.float32)
    bias_mu = np.ascontiguousarray(bias_mu, dtype=np.float32)
    bias_sigma = np.ascontiguousarray(bias_sigma, dtype=np.float32)
    eps_w = np.ascontiguousarray(eps_w, dtype=np.float32)
    eps_b = np.ascontiguousarray(eps_b, dtype=np.float32)

    nc = _get_program((S_PC, O_PC, IN, B))

    in_maps = []
    for c in range(N_CORES):
        sg, og = divmod(c, OUT_WAYS)
        s_lo, s_hi = sg * S_PC, (sg + 1) * S_PC
        o_lo, o_hi = og * O_PC, (og + 1) * O_PC
        in_maps.append({
            "x": x[s_lo:s_hi],
            "eps_w": np.ascontiguousarray(eps_w[s_lo:s_hi, o_lo:o_hi, :]),
            "mu": np.ascontiguousarray(weight_mu[o_lo:o_hi]),
            "sigma": np.ascontiguousarray(weight_sigma[o_lo:o_hi]),
            "bias_mu": bias_mu[o_lo:o_hi].reshape(1, O_PC),
            "bias_sigma": bias_sigma[o_lo:o_hi].reshape(1, O_PC),
            "eps_b": np.ascontiguousarray(eps_b[s_lo:s_hi, o_lo:o_hi]),
        })

    global _last_in_maps
    _last_in_maps = in_maps
    res = run_bass_kernel_spmd(nc, in_maps, core_ids=list(range(N_CORES)))

    out = np.empty((S, B, OUT), dtype=np.float32)
    for c in range(N_CORES):
        sg, og = divmod(c, OUT_WAYS)
        out[sg * S_PC:(sg + 1) * S_PC, :, og * O_PC:(og + 1) * O_PC] = res.results[c]["out"]
    return out
